# revision 13
# baseline (speedup 1.0000x reference)
"""Trainium2 Bass kernel for nn_Attention_5093831213465.

Reference computation (per sample, x_b: [256, 4096]):
  q = Wq @ x_b                       [32, 4096]
  k = maxpool2(Wk @ x_b)             [32, 1024]
  v = maxpool2(Wv @ x_b)             [128, 1024]
  attn = softmax_over_k(k^T @ q)     [1024, 4096]
  out  = Wa @ (v @ attn)             [256, 4096]
  y    = gamma * out + x_b
Sharding: data-parallel over batch, 2 samples per core on 8 cores.

Design (driven by the TimelineSim cost model):
- Matmul cost = out-free-size x cycles/row; fp8 DoubleRow = 0.5/row.
  The value matmul (v @ E) runs entirely in fp8 DoubleRow: E in e5m2
  written by the Act engine's exp, v^T in e4m3.
- Softmax denominators via "stationary-E": matmuls with E as the
  stationary operand and a ones column moving -> out free size 1, so
  the whole reduction costs ~nothing on the PE (vs. streaming E again).
- exp overflows e5m2 unless logits are shifted per column.  The shift
  rides the attention matmul as an extra contraction row: k row 32 is
  constant 1, q row 32 is -(submax[qq]+1), where submax is a 128-key
  subsampled column max computed by a small transposed attention
  (q-tile stationary) + a DVE free-dim max.  Measured gap between true
  colmax and 128-submax on this data is <= 8.81, safely under the
  ~12 overflow budget.
- Normalization happens pre-Wa on the DVE (un = psU * rb, e4m3 out);
  rb is built per chunk: denom -> reciprocal (bf16) -> PE transpose ->
  SBUF->SBUF partition-gather DMA -> gpsimd partition_broadcast.
  gpsimd cannot touch PSUM, so it only gets SBUF-only jobs.
- Residual adds on DVE from psO + x, one [128,2,512] instr per chunk.
- All large DMAs ride the SP queue; x loads are split so chunk 0's
  columns land first and the PE starts early.
"""

import sys

import numpy as np

if "/opt/trn_rl_repo" not in sys.path:
    sys.path.insert(0, "/opt/trn_rl_repo")

B, C, H, W = 16, 256, 64, 64
CA = C // 8          # 32  attn channels
CS = C // 2          # 128 value channels
HWF = H * W          # 4096 spatial positions
HWP = HWF // 4       # 1024 pooled positions
SPC = 2              # samples per core
NCORES = 8
CHUNK = 512
NCHUNK = HWF // CHUNK       # 8
KT = HWP // 128             # 8 kk tiles of 128
NPAIR = KT // 2             # 4 exp/U pairs per chunk
SHIFT_DELTA = 2.0           # c = submax + delta

_built = {}


def _build_program():
    from contextlib import ExitStack

    import concourse.bass as bass
    import concourse.tile as tile
    from concourse import bacc, mybir

    f32 = mybir.dt.float32
    f32r = mybir.dt.float32r
    bf16 = mybir.dt.bfloat16
    e4 = mybir.dt.float8e4
    e5 = mybir.dt.float8e5
    i16 = mybir.dt.int16
    DR = mybir.MatmulPerfMode.DoubleRow
    Exp = mybir.ActivationFunctionType.Exp
    Mult = mybir.AluOpType.mult
    Add = mybir.AluOpType.add
    Max = mybir.AluOpType.max

    nc = bacc.Bacc(
        "TRN2", target_bir_lowering=False, debug=False, enable_asserts=False
    )

    x_d = nc.dram_tensor("x", [SPC, 2, 128, HWF], f32r, kind="ExternalInput").ap()
    wqk_d = nc.dram_tensor("wqkT", [128, 2, 64], f32r, kind="ExternalInput").ap()
    wv_d = nc.dram_tensor("wvT", [128, 2, 128], f32r, kind="ExternalInput").ap()
    wa_d = nc.dram_tensor("waT", [128, 2, 128], bf16, kind="ExternalInput").ap()
    idb_d = nc.dram_tensor("identB", [128, 128], bf16, kind="ExternalInput").ap()
    idf_d = nc.dram_tensor("identF", [128, 128], f32, kind="ExternalInput").ap()
    on8_d = nc.dram_tensor("ones8", [128, 2, 1], e5, kind="ExternalInput").ap()
    kone_d = nc.dram_tensor("kone", [1, KT, 128], bf16, kind="ExternalInput").ap()
    y_d = nc.dram_tensor("y", [SPC, 2, 128, HWF], f32, kind="ExternalOutput").ap()

    with tile.TileContext(nc) as tc, ExitStack() as ctx:
        consts = ctx.enter_context(tc.tile_pool(name="consts", bufs=1))
        xp = ctx.enter_context(tc.tile_pool(name="xp", bufs=2))
        qsp = ctx.enter_context(tc.tile_pool(name="qsp", bufs=2))
        kvp = ctx.enter_context(tc.tile_pool(name="kvp", bufs=2))
        cm = ctx.enter_context(tc.tile_pool(name="cm", bufs=2))
        ep = ctx.enter_context(tc.tile_pool(name="ep", bufs=6))
        rp = ctx.enter_context(tc.tile_pool(name="rp", bufs=3))
        up = ctx.enter_context(tc.tile_pool(name="up", bufs=3))
        yp = ctx.enter_context(tc.tile_pool(name="yp", bufs=3))
        # PSUM budget (16KB/partition): pBig 2x[128,2,512]f32 (8KB) shared by
        # conv tiles and attn pairs (disjoint in time), pW 3x[128,512]f32 (6KB)
        # for U/rb-chain/Wa outputs and small transposes.
        pBig = ctx.enter_context(tc.tile_pool(name="pBig", bufs=2, space="PSUM"))
        pW = ctx.enter_context(tc.tile_pool(name="pW", bufs=3, space="PSUM"))

        wqk = consts.tile([128, 2, 64], f32r)
        nc.sync.dma_start(wqk[:], wqk_d)
        wv = consts.tile([128, 2, 128], f32r)
        nc.sync.dma_start(wv[:], wv_d)
        wa = consts.tile([128, 2, 128], bf16)
        nc.sync.dma_start(wa[:], wa_d)
        idb = consts.tile([128, 128], bf16)
        nc.sync.dma_start(idb[:], idb_d)
        idf = consts.tile([128, 128], f32)
        nc.sync.dma_start(idf[:], idf_d)
        on8 = consts.tile([128, 2, 1], e5)
        nc.sync.dma_start(on8[:], on8_d)

        # x loads: front chunk first so conv starts early
        xrs = []
        for s in range(SPC):
            xr = xp.tile([128, 2, HWF], f32r, tag="xr")
            xrs.append(xr)
            for lo, hi in ((0, 512), (512, 2048), (2048, 4096)):
                for t in range(2):
                    nc.sync.dma_start(
                        xr[:, t, lo:hi], x_d[s, t, :, lo:hi]
                    )

        qs_l, kph_l, vT_l, cneg_l = [], [], [], []

        # ---- conv + pool + submax phases (both samples before attn) ----
        for s in range(SPC):
            qs = qsp.tile([33, KT, CHUNK], bf16, tag="qs")
            kph = kvp.tile([33, KT, 128], bf16, tag="kph")
            vph = kvp.tile([128, KT, 128], bf16, tag="vph")
            vT = kvp.tile([128, NPAIR, 2, 128], e4, tag="vT")
            qs_l.append(qs)
            kph_l.append(kph)
            vT_l.append(vT)

            # k-side ones row for the shift
            nc.sync.dma_start(kph[32:33, :, :], kone_d)

            for ck in range(NCHUNK):
                cs = slice(ck * CHUNK, (ck + 1) * CHUNK)
                pcv = pBig.tile([128, 2, CHUNK], f32, tag="big")
                for t in range(2):
                    nc.tensor.matmul(
                        pcv[0:64, 0, :], wqk[:, t, :], xrs[s][:, t, cs],
                        start=(t == 0), stop=(t == 1),
                    )
                nc.vector.tensor_copy(qs[0:32, ck, :], pcv[0:32, 0, :])
                nc.vector.tensor_reduce(
                    kph[0:32, ck, :].rearrange("p (h2 w2) -> p h2 w2", h2=4),
                    pcv[32:64, 0, :].rearrange(
                        "p (h2 dh w2 dw) -> p h2 w2 dh dw", h2=4, dh=2, w2=32, dw=2
                    ),
                    axis=mybir.AxisListType.XY, op=Max,
                )
                for t in range(2):
                    nc.tensor.matmul(
                        pcv[:, 1, :], wv[:, t, :], xrs[s][:, t, cs],
                        start=(t == 0), stop=(t == 1),
                    )
                nc.vector.tensor_reduce(
                    vph[:, ck, :].rearrange("p (h2 w2) -> p h2 w2", h2=4),
                    pcv[:, 1, :].rearrange(
                        "p (h2 dh w2 dw) -> p h2 w2 dh dw", h2=4, dh=2, w2=32, dw=2
                    ),
                    axis=mybir.AxisListType.XY, op=Max,
                )
                ptr = pW.tile([128, 128], bf16, tag="w")
                nc.tensor.transpose(ptr[:], vph[:, ck, :], idb[:])
                nc.scalar.copy(vT[:, ck // 2, ck % 2, :], ptr[:])

            # submax: transposed 128-key subsampled attention + free max.
            # bf16 moving operand: f32r would pay the 4x short-row penalty
            # on the [*, 128] outputs.
            ksub = kph[0:32, :, :].rearrange(
                "p kt (j v) -> p kt j v", v=16
            )[:, :, :, 0]
            cmax = cm.tile([128, 32], f32r, tag="cmax")
            for ck in range(NCHUNK):
                psm = pBig.tile([128, 4, 64], f32, tag="big")
                for j in range(4):
                    nc.tensor.matmul(
                        psm[:, j, :],
                        qs[0:32, ck, j * 128 : (j + 1) * 128],
                        ksub,
                        start=True, stop=True,
                    )
                nc.vector.tensor_reduce(
                    cmax[:, ck * 4 : ck * 4 + 4],
                    psm[:],
                    axis=mybir.AxisListType.X, op=Max,
                )
            cneg = cm.tile([128, 32], bf16, tag="cneg")
            nc.vector.tensor_scalar(
                cneg[:], cmax[:], -1.0, -SHIFT_DELTA, Mult, Add
            )
            pcn = pW.tile([32, 128], bf16, tag="w")
            nc.tensor.transpose(pcn[:], cneg[:], idb[:])
            cnT = cm.tile([32, 128], bf16, tag="cnT")
            nc.vector.tensor_copy(cnT[:], pcn[:])
            cneg_l.append(cnT)
            # scatter the q shift row: [32,128] partitions -> [1, 8, 512]
            nc.gpsimd.dma_start(
                qs[32:33, :, :].rearrange("o kt (j m) -> o (kt j) m", j=4),
                cnT[:],
            )

        # ---- attention phases ----
        for s in range(SPC):
            qs, kph, vT = qs_l[s], kph_l[s], vT_l[s]
            for ck in range(NCHUNK):
                cs = slice(ck * CHUNK, (ck + 1) * CHUNK)
                egs = []
                for g in range(NPAIR):
                    pa = pBig.tile([128, 2, CHUNK], f32, tag="big")
                    for i in range(2):
                        nc.tensor.matmul(
                            pa[:, i, :],
                            kph[:, 2 * g + i, :],
                            qs[:, ck, :],
                            start=True, stop=True,
                        )
                    eg = ep.tile([128, 2, CHUNK], e5, tag="E")
                    nc.scalar.activation(eg[:], pa[:], Exp)
                    egs.append(eg)

                # denominators: stationary-E DoubleRow, ones moving
                den = pW.tile([128, 4], f32, tag="w")
                for j in range(4):
                    for g in range(NPAIR):
                        nc.tensor.matmul(
                            den[:, j : j + 1],
                            egs[g][:, :, j * 128 : (j + 1) * 128],
                            on8[:],
                            start=(g == 0), stop=(g == NPAIR - 1),
                            perf_mode=DR,
                        )
                r4 = rp.tile([128, 4], f32, tag="r4")
                nc.vector.reciprocal_approx_fast(r4[:], den[:])
                prT = pW.tile([4, 128], f32, tag="w")
                nc.tensor.transpose(prT[:], r4[:], idf[:])
                rr4 = rp.tile([4, 128], f32, tag="rr4")
                nc.vector.tensor_copy(rr4[:], prT[:])
                rrow = rp.tile([1, CHUNK], f32, tag="rrow")
                nc.gpsimd.dma_start(
                    rrow[0:1, :].rearrange("o (j m) -> o j m", j=4), rr4[:]
                )
                rb = rp.tile([128, CHUNK], f32, tag="rb")
                nc.gpsimd.partition_broadcast(rb[:], rrow[0:1, :])

                pu = pW.tile([128, CHUNK], f32, tag="w")
                for g in range(NPAIR):
                    nc.tensor.matmul(
                        pu[:], vT[:, g, :, :], egs[g][:],
                        start=(g == 0), stop=(g == NPAIR - 1),
                        perf_mode=DR,
                    )
                un = up.tile([128, CHUNK], e4, tag="un")
                nc.vector.tensor_mul(un[:], pu[:], rb[:])

                yt = yp.tile([128, 2, CHUNK], f32, tag="y")
                for mt in range(2):
                    po = pW.tile([128, CHUNK], f32, tag="w")
                    nc.tensor.matmul(
                        po[:], wa[:, mt, :], un[:],
                        start=True, stop=True,
                    )
                    nc.vector.tensor_add(
                        yt[:, mt, :], po[:], xrs[s][:, mt, cs].bitcast(f32)
                    )
                nc.gpsimd.dma_start(y_d[s, :, :, cs].rearrange("t p m -> p t m"), yt[:])

    nc.compile()
    return nc


def _get_program():
    if "nc" not in _built:
        _built["nc"] = _build_program()
    return _built["nc"]


def _make_in_maps(x, Wq, Wk, Wv, Wa, gamma):
    import ml_dtypes

    x = np.ascontiguousarray(
        np.asarray(x, dtype=np.float32).reshape(B, 2, 128, HWF)
    )
    wqkT = np.concatenate([np.asarray(Wq), np.asarray(Wk)], axis=0).T
    wqkT = np.ascontiguousarray(
        wqkT.reshape(2, 128, 64).transpose(1, 0, 2).astype(np.float32)
    )
    wvT = np.ascontiguousarray(
        np.asarray(Wv).T.reshape(2, 128, 128).transpose(1, 0, 2).astype(np.float32)
    )
    g = float(np.asarray(gamma).reshape(-1)[0])
    waT = np.ascontiguousarray(
        (g * np.asarray(Wa)).T.reshape(128, 2, 128).astype(ml_dtypes.bfloat16)
    )
    identB = np.eye(128, dtype=np.float32).astype(ml_dtypes.bfloat16)
    identF = np.eye(128, dtype=np.float32)
    ones8 = np.ones((128, 2, 1), dtype=ml_dtypes.float8_e5m2)
    kone = np.ones((1, KT, 128), dtype=ml_dtypes.bfloat16)
    return [
        {
            "x": np.ascontiguousarray(x[c * SPC : (c + 1) * SPC]),
            "wqkT": wqkT,
            "wvT": wvT,
            "waT": waT,
            "identB": identB,
            "identF": identF,
            "ones8": ones8,
            "kone": kone,
        }
        for c in range(NCORES)
    ]


def kernel(x, Wq, Wk, Wv, Wa, gamma):
    from concourse import bass_utils

    nc = _get_program()
    in_maps = _make_in_maps(x, Wq, Wk, Wv, Wa, gamma)
    res = bass_utils.run_bass_kernel_spmd(
        nc, in_maps, core_ids=list(range(NCORES))
    )
    out = np.concatenate(
        [res.results[c]["y"].reshape(1, SPC, C, HWF) for c in range(NCORES)],
        axis=0,
    ).reshape(B, C, H, W)
    return out


# revision 15
# speedup vs baseline: 1.0444x; 1.0444x over previous
"""Trainium2 Bass kernel for nn_Attention_5093831213465.

Reference computation (per sample, x_b: [256, 4096]):
  q = Wq @ x_b                       [32, 4096]
  k = maxpool2(Wk @ x_b)             [32, 1024]
  v = maxpool2(Wv @ x_b)             [128, 1024]
  attn = softmax_over_k(k^T @ q)     [1024, 4096]
  out  = Wa @ (v @ attn)             [256, 4096]
  y    = gamma * out + x_b
Sharding: data-parallel over batch, 2 samples per core on 8 cores.

Design (driven by the TimelineSim cost model):
- Matmul cost = out-free-size x cycles/row; fp8 DoubleRow = 0.5/row.
  The value matmul (v @ E) runs entirely in fp8 DoubleRow: E in e5m2
  written by the Act engine's exp, v^T in e4m3.
- Softmax denominators via "stationary-E": matmuls with E as the
  stationary operand and a ones column moving -> out free size 1, so
  the whole reduction costs ~nothing on the PE (vs. streaming E again).
- exp overflows e5m2 unless logits are shifted per column.  The shift
  rides the attention matmul as an extra contraction row: k row 32 is
  constant 1, q row 32 is -(submax[qq]+1), where submax is a 128-key
  subsampled column max computed by a small transposed attention
  (q-tile stationary) + a DVE free-dim max.  Measured gap between true
  colmax and 128-submax on this data is <= 8.81, safely under the
  ~12 overflow budget.
- Normalization happens pre-Wa on the DVE (un = psU * rb, e4m3 out);
  rb is built per chunk: denom -> reciprocal (bf16) -> PE transpose ->
  SBUF->SBUF partition-gather DMA -> gpsimd partition_broadcast.
  gpsimd cannot touch PSUM, so it only gets SBUF-only jobs.
- Residual adds on DVE from psO + x, one [128,2,512] instr per chunk.
- All large DMAs ride the SP queue; x loads are split so chunk 0's
  columns land first and the PE starts early.
"""

import sys

import numpy as np

if "/opt/trn_rl_repo" not in sys.path:
    sys.path.insert(0, "/opt/trn_rl_repo")

B, C, H, W = 16, 256, 64, 64
CA = C // 8          # 32  attn channels
CS = C // 2          # 128 value channels
HWF = H * W          # 4096 spatial positions
HWP = HWF // 4       # 1024 pooled positions
SPC = 2              # samples per core
NCORES = 8
CHUNK = 512
NCHUNK = HWF // CHUNK       # 8
KT = HWP // 128             # 8 kk tiles of 128
NPAIR = KT // 2             # 4 exp/U pairs per chunk
SHIFT_DELTA = 2.0           # c = submax + delta

_built = {}


def _build_program():
    from contextlib import ExitStack

    import concourse.bass as bass
    import concourse.tile as tile
    from concourse import bacc, mybir

    f32 = mybir.dt.float32
    f32r = mybir.dt.float32r
    bf16 = mybir.dt.bfloat16
    e4 = mybir.dt.float8e4
    e5 = mybir.dt.float8e5
    i16 = mybir.dt.int16
    DR = mybir.MatmulPerfMode.DoubleRow
    Exp = mybir.ActivationFunctionType.Exp
    Mult = mybir.AluOpType.mult
    Add = mybir.AluOpType.add
    Max = mybir.AluOpType.max

    nc = bacc.Bacc(
        "TRN2", target_bir_lowering=False, debug=False, enable_asserts=False
    )

    x_d = nc.dram_tensor("x", [SPC, 2, 128, HWF], f32r, kind="ExternalInput").ap()
    wqk_d = nc.dram_tensor("wqkT", [128, 2, 64], f32r, kind="ExternalInput").ap()
    wv_d = nc.dram_tensor("wvT", [128, 2, 128], f32r, kind="ExternalInput").ap()
    wa_d = nc.dram_tensor("waT", [128, 2, 128], bf16, kind="ExternalInput").ap()
    idb_d = nc.dram_tensor("identB", [128, 128], bf16, kind="ExternalInput").ap()
    idf_d = nc.dram_tensor("identF", [128, 128], f32, kind="ExternalInput").ap()
    on8_d = nc.dram_tensor("ones8", [128, 2, 1], e5, kind="ExternalInput").ap()
    kone_d = nc.dram_tensor("kone", [1, KT, 128], bf16, kind="ExternalInput").ap()
    y_d = nc.dram_tensor("y", [SPC, 2, 128, HWF], f32, kind="ExternalOutput").ap()

    with tile.TileContext(nc) as tc, ExitStack() as ctx:
        consts = ctx.enter_context(tc.tile_pool(name="consts", bufs=1))
        xp = ctx.enter_context(tc.tile_pool(name="xp", bufs=2))
        qsp = ctx.enter_context(tc.tile_pool(name="qsp", bufs=2))
        kvp = ctx.enter_context(tc.tile_pool(name="kvp", bufs=2))
        cm = ctx.enter_context(tc.tile_pool(name="cm", bufs=2))
        ep = ctx.enter_context(tc.tile_pool(name="ep", bufs=6))
        rp = ctx.enter_context(tc.tile_pool(name="rp", bufs=3))
        up = ctx.enter_context(tc.tile_pool(name="up", bufs=3))
        yp = ctx.enter_context(tc.tile_pool(name="yp", bufs=3))
        # PSUM budget (16KB/partition): pBig 2x[128,2,512]f32 (8KB) shared by
        # conv tiles and attn pairs (disjoint in time), pW 3x[128,512]f32 (6KB)
        # for U/rb-chain/Wa outputs and small transposes.
        pBig = ctx.enter_context(tc.tile_pool(name="pBig", bufs=2, space="PSUM"))
        pW = ctx.enter_context(tc.tile_pool(name="pW", bufs=4, space="PSUM"))

        wqk = consts.tile([128, 2, 64], f32r)
        nc.sync.dma_start(wqk[:], wqk_d)
        wv = consts.tile([128, 2, 128], f32r)
        nc.sync.dma_start(wv[:], wv_d)
        wa = consts.tile([128, 2, 128], bf16)
        nc.sync.dma_start(wa[:], wa_d)
        idb = consts.tile([128, 128], bf16)
        nc.sync.dma_start(idb[:], idb_d)
        idf = consts.tile([128, 128], f32)
        nc.sync.dma_start(idf[:], idf_d)
        on8 = consts.tile([128, 2, 1], e5)
        nc.sync.dma_start(on8[:], on8_d)

        # x loads: front chunk first so conv starts early
        xrs = []
        for s in range(SPC):
            xr = xp.tile([128, 2, HWF], f32r, tag="xr")
            xrs.append(xr)
            for lo, hi in ((0, 512), (512, 2048), (2048, 4096)):
                for t in range(2):
                    nc.sync.dma_start(
                        xr[:, t, lo:hi], x_d[s, t, :, lo:hi]
                    )

        qs_l, kph_l, vT_l, cneg_l = [], [], [], []

        # ---- conv + pool + submax phases (both samples before attn) ----
        for s in range(SPC):
            qs = qsp.tile([33, KT, CHUNK], bf16, tag="qs")
            kph = kvp.tile([33, KT, 128], bf16, tag="kph")
            vph = kvp.tile([128, KT, 128], bf16, tag="vph")
            vT = kvp.tile([128, NPAIR, 2, 128], e4, tag="vT")
            qs_l.append(qs)
            kph_l.append(kph)
            vT_l.append(vT)

            # k-side ones row for the shift
            nc.sync.dma_start(kph[32:33, :, :], kone_d)

            for ck in range(NCHUNK):
                cs = slice(ck * CHUNK, (ck + 1) * CHUNK)
                pcv = pBig.tile([128, 2, CHUNK], f32, tag="big")
                for t in range(2):
                    nc.tensor.matmul(
                        pcv[0:64, 0, :], wqk[:, t, :], xrs[s][:, t, cs],
                        start=(t == 0), stop=(t == 1),
                    )
                nc.vector.tensor_copy(qs[0:32, ck, :], pcv[0:32, 0, :])
                nc.vector.tensor_reduce(
                    kph[0:32, ck, :].rearrange("p (h2 w2) -> p h2 w2", h2=4),
                    pcv[32:64, 0, :].rearrange(
                        "p (h2 dh w2 dw) -> p h2 w2 dh dw", h2=4, dh=2, w2=32, dw=2
                    ),
                    axis=mybir.AxisListType.XY, op=Max,
                )
                for t in range(2):
                    nc.tensor.matmul(
                        pcv[:, 1, :], wv[:, t, :], xrs[s][:, t, cs],
                        start=(t == 0), stop=(t == 1),
                    )
                nc.vector.tensor_reduce(
                    vph[:, ck, :].rearrange("p (h2 w2) -> p h2 w2", h2=4),
                    pcv[:, 1, :].rearrange(
                        "p (h2 dh w2 dw) -> p h2 w2 dh dw", h2=4, dh=2, w2=32, dw=2
                    ),
                    axis=mybir.AxisListType.XY, op=Max,
                )
                ptr = pW.tile([128, 128], bf16, tag="w")
                nc.tensor.transpose(ptr[:], vph[:, ck, :], idb[:])
                nc.scalar.copy(vT[:, ck // 2, ck % 2, :], ptr[:])

            # submax: transposed 128-key subsampled attention + free max.
            # bf16 moving operand: f32r would pay the 4x short-row penalty
            # on the [*, 128] outputs.
            ksub = kph[0:32, :, :].rearrange(
                "p kt (j v) -> p kt j v", v=16
            )[:, :, :, 0]
            cmax = cm.tile([128, 32], f32r, tag="cmax")
            for ck in range(NCHUNK):
                psm = pBig.tile([128, 4, 64], f32, tag="big")
                for j in range(4):
                    nc.tensor.matmul(
                        psm[:, j, :],
                        qs[0:32, ck, j * 128 : (j + 1) * 128],
                        ksub,
                        start=True, stop=True,
                    )
                nc.vector.tensor_reduce(
                    cmax[:, ck * 4 : ck * 4 + 4],
                    psm[:],
                    axis=mybir.AxisListType.X, op=Max,
                )
            cneg = cm.tile([128, 32], bf16, tag="cneg")
            nc.vector.tensor_scalar(
                cneg[:], cmax[:], -1.0, -SHIFT_DELTA, Mult, Add
            )
            pcn = pW.tile([32, 128], bf16, tag="w")
            nc.tensor.transpose(pcn[:], cneg[:], idb[:])
            cnT = cm.tile([32, 128], bf16, tag="cnT")
            nc.vector.tensor_copy(cnT[:], pcn[:])
            cneg_l.append(cnT)
            # scatter the q shift row: [32,128] partitions -> [1, 8, 512]
            nc.gpsimd.dma_start(
                qs[32:33, :, :].rearrange("o kt (j m) -> o (kt j) m", j=4),
                cnT[:],
            )

        # ---- attention phases ----
        for s in range(SPC):
            qs, kph, vT = qs_l[s], kph_l[s], vT_l[s]
            for ck in range(NCHUNK):
                cs = slice(ck * CHUNK, (ck + 1) * CHUNK)
                egs = []
                for g in range(NPAIR):
                    pa = pBig.tile([128, 2, CHUNK], f32, tag="big")
                    for i in range(2):
                        nc.tensor.matmul(
                            pa[:, i, :],
                            kph[:, 2 * g + i, :],
                            qs[:, ck, :],
                            start=True, stop=True,
                        )
                    eg = ep.tile([128, 2, CHUNK], e5, tag="E")
                    nc.scalar.activation(eg[:], pa[:], Exp)
                    egs.append(eg)

                # denominators: stationary-E DoubleRow, ones moving
                den = pW.tile([128, 4], f32, tag="w")
                for j in range(4):
                    for g in range(NPAIR):
                        nc.tensor.matmul(
                            den[:, j : j + 1],
                            egs[g][:, :, j * 128 : (j + 1) * 128],
                            on8[:],
                            start=(g == 0), stop=(g == NPAIR - 1),
                            perf_mode=DR,
                        )
                r4 = rp.tile([128, 4], f32, tag="r4")
                nc.vector.reciprocal_approx_fast(r4[:], den[:])
                prT = pW.tile([4, 128], f32, tag="w")
                nc.tensor.transpose(prT[:], r4[:], idf[:])
                rr4 = rp.tile([4, 128], f32, tag="rr4")
                nc.vector.tensor_copy(rr4[:], prT[:])
                rrow = rp.tile([1, CHUNK], f32, tag="rrow")
                nc.gpsimd.dma_start(
                    rrow[0:1, :].rearrange("o (j m) -> o j m", j=4), rr4[:]
                )
                rb = rp.tile([128, CHUNK], f32, tag="rb")
                nc.gpsimd.partition_broadcast(rb[:], rrow[0:1, :])

                pu = pW.tile([128, CHUNK], f32, tag="w")
                for g in range(NPAIR):
                    nc.tensor.matmul(
                        pu[:], vT[:, g, :, :], egs[g][:],
                        start=(g == 0), stop=(g == NPAIR - 1),
                        perf_mode=DR,
                    )
                un = up.tile([128, CHUNK], e4, tag="un")
                nc.vector.tensor_mul(un[:], pu[:], rb[:])

                yt = yp.tile([128, 2, CHUNK], f32, tag="y")
                for mt in range(2):
                    po = pW.tile([128, CHUNK], f32, tag="w")
                    nc.tensor.matmul(
                        po[:], wa[:, mt, :], un[:],
                        start=True, stop=True,
                    )
                    nc.vector.tensor_add(
                        yt[:, mt, :], po[:], xrs[s][:, mt, cs].bitcast(f32)
                    )
                nc.sync.dma_start(y_d[s, :, :, cs].rearrange("t p m -> p t m"), yt[:])

    nc.compile()
    return nc


def _get_program():
    if "nc" not in _built:
        _built["nc"] = _build_program()
    return _built["nc"]


def _make_in_maps(x, Wq, Wk, Wv, Wa, gamma):
    import ml_dtypes

    x = np.ascontiguousarray(
        np.asarray(x, dtype=np.float32).reshape(B, 2, 128, HWF)
    )
    wqkT = np.concatenate([np.asarray(Wq), np.asarray(Wk)], axis=0).T
    wqkT = np.ascontiguousarray(
        wqkT.reshape(2, 128, 64).transpose(1, 0, 2).astype(np.float32)
    )
    wvT = np.ascontiguousarray(
        np.asarray(Wv).T.reshape(2, 128, 128).transpose(1, 0, 2).astype(np.float32)
    )
    g = float(np.asarray(gamma).reshape(-1)[0])
    waT = np.ascontiguousarray(
        (g * np.asarray(Wa)).T.reshape(128, 2, 128).astype(ml_dtypes.bfloat16)
    )
    identB = np.eye(128, dtype=np.float32).astype(ml_dtypes.bfloat16)
    identF = np.eye(128, dtype=np.float32)
    ones8 = np.ones((128, 2, 1), dtype=ml_dtypes.float8_e5m2)
    kone = np.ones((1, KT, 128), dtype=ml_dtypes.bfloat16)
    return [
        {
            "x": np.ascontiguousarray(x[c * SPC : (c + 1) * SPC]),
            "wqkT": wqkT,
            "wvT": wvT,
            "waT": waT,
            "identB": identB,
            "identF": identF,
            "ones8": ones8,
            "kone": kone,
        }
        for c in range(NCORES)
    ]


def kernel(x, Wq, Wk, Wv, Wa, gamma):
    from concourse import bass_utils

    nc = _get_program()
    in_maps = _make_in_maps(x, Wq, Wk, Wv, Wa, gamma)
    res = bass_utils.run_bass_kernel_spmd(
        nc, in_maps, core_ids=list(range(NCORES))
    )
    out = np.concatenate(
        [res.results[c]["y"].reshape(1, SPC, C, HWF) for c in range(NCORES)],
        axis=0,
    ).reshape(B, C, H, W)
    return out


# revision 16
# speedup vs baseline: 1.3559x; 1.2983x over previous
"""Trainium2 Bass kernel for nn_Attention_5093831213465.

Reference computation (per sample, x_b: [256, 4096]):
  q = Wq @ x_b                       [32, 4096]
  k = maxpool2(Wk @ x_b)             [32, 1024]
  v = maxpool2(Wv @ x_b)             [128, 1024]
  attn = softmax_over_k(k^T @ q)     [1024, 4096]
  out  = Wa @ (v @ attn)             [256, 4096]
  y    = gamma * out + x_b
Sharding: data-parallel over batch, 2 samples per core on 8 cores.

Design (driven by the TimelineSim cost model):
- Matmul cost = out-free-size x cycles/row; fp8 DoubleRow = 0.5/row.
  The value matmul (v @ E) runs entirely in fp8 DoubleRow: E in e5m2
  written by the Act engine's exp, v^T in e4m3.
- Softmax denominators via "stationary-E": matmuls with E as the
  stationary operand and a ones column moving -> out free size 1, so
  the whole reduction costs ~nothing on the PE (vs. streaming E again).
- exp overflows e5m2 unless logits are shifted per column.  The shift
  rides the attention matmul as an extra contraction row: k row 32 is
  constant 1, q row 32 is -(submax[qq]+1), where submax is a 128-key
  subsampled column max computed by a small transposed attention
  (q-tile stationary) + a DVE free-dim max.  Measured gap between true
  colmax and 128-submax on this data is <= 8.81, safely under the
  ~12 overflow budget.
- Normalization happens pre-Wa on the DVE (un = psU * rb, e4m3 out);
  rb is built per chunk: denom -> reciprocal (bf16) -> PE transpose ->
  SBUF->SBUF partition-gather DMA -> gpsimd partition_broadcast.
  gpsimd cannot touch PSUM, so it only gets SBUF-only jobs.
- Residual adds on DVE from psO + x, one [128,2,512] instr per chunk.
- All large DMAs ride the SP queue; x loads are split so chunk 0's
  columns land first and the PE starts early.
"""

import sys

import numpy as np

if "/opt/trn_rl_repo" not in sys.path:
    sys.path.insert(0, "/opt/trn_rl_repo")

B, C, H, W = 16, 256, 64, 64
CA = C // 8          # 32  attn channels
CS = C // 2          # 128 value channels
HWF = H * W          # 4096 spatial positions
HWP = HWF // 4       # 1024 pooled positions
SPC = 2              # samples per core
NCORES = 8
CHUNK = 512
NCHUNK = HWF // CHUNK       # 8
KT = HWP // 128             # 8 kk tiles of 128
NPAIR = KT // 2             # 4 exp/U pairs per chunk
SHIFT_DELTA = 2.0           # c = submax + delta

_built = {}


def _build_program():
    from contextlib import ExitStack

    import concourse.bass as bass
    import concourse.tile as tile
    from concourse import bacc, mybir

    f32 = mybir.dt.float32
    f32r = mybir.dt.float32r
    bf16 = mybir.dt.bfloat16
    e4 = mybir.dt.float8e4
    e5 = mybir.dt.float8e5
    i16 = mybir.dt.int16
    DR = mybir.MatmulPerfMode.DoubleRow
    Exp = mybir.ActivationFunctionType.Exp
    Mult = mybir.AluOpType.mult
    Add = mybir.AluOpType.add
    Max = mybir.AluOpType.max

    nc = bacc.Bacc(
        "TRN2", target_bir_lowering=False, debug=False, enable_asserts=False
    )

    x_d = nc.dram_tensor("x", [SPC, 2, 128, HWF], f32r, kind="ExternalInput").ap()
    wqk_d = nc.dram_tensor("wqkT", [128, 2, 64], f32r, kind="ExternalInput").ap()
    wv_d = nc.dram_tensor("wvT", [128, 2, 128], f32r, kind="ExternalInput").ap()
    wa_d = nc.dram_tensor("waT", [128, 2, 128], bf16, kind="ExternalInput").ap()
    idb_d = nc.dram_tensor("identB", [128, 128], bf16, kind="ExternalInput").ap()
    idf_d = nc.dram_tensor("identF", [128, 128], f32, kind="ExternalInput").ap()
    on8_d = nc.dram_tensor("ones8", [128, 2, 1], e5, kind="ExternalInput").ap()
    kone_d = nc.dram_tensor("kone", [1, KT, 128], bf16, kind="ExternalInput").ap()
    y_d = nc.dram_tensor("y", [SPC, 2, 128, HWF], f32, kind="ExternalOutput").ap()

    with tile.TileContext(nc) as tc, ExitStack() as ctx:
        consts = ctx.enter_context(tc.tile_pool(name="consts", bufs=1))
        xp = ctx.enter_context(tc.tile_pool(name="xp", bufs=2))
        qsp = ctx.enter_context(tc.tile_pool(name="qsp", bufs=2))
        kvp = ctx.enter_context(tc.tile_pool(name="kvp", bufs=2))
        cm = ctx.enter_context(tc.tile_pool(name="cm", bufs=2))
        ep = ctx.enter_context(tc.tile_pool(name="ep", bufs=12))
        rp = ctx.enter_context(tc.tile_pool(name="rp", bufs=3))
        up = ctx.enter_context(tc.tile_pool(name="up", bufs=3))
        yp = ctx.enter_context(tc.tile_pool(name="yp", bufs=3))
        # PSUM budget (16KB/partition): pBig 2x[128,2,512]f32 (8KB) shared by
        # conv tiles and attn pairs (disjoint in time), pW 3x[128,512]f32 (6KB)
        # for U/rb-chain/Wa outputs and small transposes.
        pBig = ctx.enter_context(tc.tile_pool(name="pBig", bufs=2, space="PSUM"))
        pWu = ctx.enter_context(tc.tile_pool(name="pWu", bufs=2, space="PSUM"))
        pWo = ctx.enter_context(tc.tile_pool(name="pWo", bufs=1, space="PSUM"))
        pWsm = ctx.enter_context(tc.tile_pool(name="pWsm", bufs=1, space="PSUM"))

        wqk = consts.tile([128, 2, 64], f32r)
        nc.sync.dma_start(wqk[:], wqk_d)
        wv = consts.tile([128, 2, 128], f32r)
        nc.sync.dma_start(wv[:], wv_d)
        wa = consts.tile([128, 2, 128], bf16)
        nc.sync.dma_start(wa[:], wa_d)
        idb = consts.tile([128, 128], bf16)
        nc.sync.dma_start(idb[:], idb_d)
        idf = consts.tile([128, 128], f32)
        nc.sync.dma_start(idf[:], idf_d)
        on8 = consts.tile([128, 2, 1], e5)
        nc.sync.dma_start(on8[:], on8_d)

        # x loads: front chunk first so conv starts early
        xrs = []
        for s in range(SPC):
            xr = xp.tile([128, 2, HWF], f32r, tag="xr")
            xrs.append(xr)
            for lo, hi in ((0, 512), (512, 2048), (2048, 4096)):
                for t in range(2):
                    nc.sync.dma_start(
                        xr[:, t, lo:hi], x_d[s, t, :, lo:hi]
                    )

        qs_l, kph_l, vT_l, cneg_l = [], [], [], []

        # ---- conv + pool + submax phases (both samples before attn) ----
        for s in range(SPC):
            qs = qsp.tile([33, KT, CHUNK], bf16, tag="qs")
            kph = kvp.tile([33, KT, 128], bf16, tag="kph")
            vph = kvp.tile([128, KT, 128], bf16, tag="vph")
            vT = kvp.tile([128, NPAIR, 2, 128], e4, tag="vT")
            qs_l.append(qs)
            kph_l.append(kph)
            vT_l.append(vT)

            # k-side ones row for the shift
            nc.sync.dma_start(kph[32:33, :, :], kone_d)

            for ck in range(NCHUNK):
                cs = slice(ck * CHUNK, (ck + 1) * CHUNK)
                pcv = pBig.tile([128, 2, CHUNK], f32, tag="big")
                for t in range(2):
                    nc.tensor.matmul(
                        pcv[0:64, 0, :], wqk[:, t, :], xrs[s][:, t, cs],
                        start=(t == 0), stop=(t == 1),
                    )
                nc.vector.tensor_copy(qs[0:32, ck, :], pcv[0:32, 0, :])
                nc.vector.tensor_reduce(
                    kph[0:32, ck, :].rearrange("p (h2 w2) -> p h2 w2", h2=4),
                    pcv[32:64, 0, :].rearrange(
                        "p (h2 dh w2 dw) -> p h2 w2 dh dw", h2=4, dh=2, w2=32, dw=2
                    ),
                    axis=mybir.AxisListType.XY, op=Max,
                )
                for t in range(2):
                    nc.tensor.matmul(
                        pcv[:, 1, :], wv[:, t, :], xrs[s][:, t, cs],
                        start=(t == 0), stop=(t == 1),
                    )
                nc.vector.tensor_reduce(
                    vph[:, ck, :].rearrange("p (h2 w2) -> p h2 w2", h2=4),
                    pcv[:, 1, :].rearrange(
                        "p (h2 dh w2 dw) -> p h2 w2 dh dw", h2=4, dh=2, w2=32, dw=2
                    ),
                    axis=mybir.AxisListType.XY, op=Max,
                )
                ptr = pWsm.tile([128, 128], bf16, tag="sm")
                nc.tensor.transpose(ptr[:], vph[:, ck, :], idb[:])
                nc.scalar.copy(vT[:, ck // 2, ck % 2, :], ptr[:])

            # submax: transposed 128-key subsampled attention + free max.
            # bf16 moving operand: f32r would pay the 4x short-row penalty
            # on the [*, 128] outputs.
            ksub = kph[0:32, :, :].rearrange(
                "p kt (j v) -> p kt j v", v=16
            )[:, :, :, 0]
            cmax = cm.tile([128, 32], f32r, tag="cmax")
            for ck in range(NCHUNK):
                psm = pBig.tile([128, 4, 64], f32, tag="big")
                for j in range(4):
                    nc.tensor.matmul(
                        psm[:, j, :],
                        qs[0:32, ck, j * 128 : (j + 1) * 128],
                        ksub,
                        start=True, stop=True,
                    )
                nc.vector.tensor_reduce(
                    cmax[:, ck * 4 : ck * 4 + 4],
                    psm[:],
                    axis=mybir.AxisListType.X, op=Max,
                )
            cneg = cm.tile([128, 32], bf16, tag="cneg")
            nc.vector.tensor_scalar(
                cneg[:], cmax[:], -1.0, -SHIFT_DELTA, Mult, Add
            )
            pcn = pWsm.tile([32, 128], bf16, tag="sm")
            nc.tensor.transpose(pcn[:], cneg[:], idb[:])
            cnT = cm.tile([32, 128], bf16, tag="cnT")
            nc.vector.tensor_copy(cnT[:], pcn[:])
            cneg_l.append(cnT)
            # scatter the q shift row: [32,128] partitions -> [1, 8, 512]
            nc.gpsimd.dma_start(
                qs[32:33, :, :].rearrange("o kt (j m) -> o (kt j) m", j=4),
                cnT[:],
            )

        # ---- attention phases: 2-chunk software pipeline ----
        # PE executes in order, so chunk tails (denominator chain, U, Wa)
        # that wait on the Act exp stream are emitted two chunks behind the
        # attention pair matmuls; the PE never blocks on exp.
        jobs = [(s, ck) for s in range(SPC) for ck in range(NCHUNK)]
        LAG = 2
        pend = {}

        def emit_head(i):
            s, ck = jobs[i]
            qs, kph = qs_l[s], kph_l[s]
            egs = []
            for g in range(NPAIR):
                pa = pBig.tile([128, 2, CHUNK], f32, tag="big")
                for t in range(2):
                    nc.tensor.matmul(
                        pa[:, t, :],
                        kph[:, 2 * g + t, :],
                        qs[:, ck, :],
                        start=True, stop=True,
                    )
                eg = ep.tile([128, 2, CHUNK], e5, tag="E")
                nc.scalar.activation(eg[:], pa[:], Exp)
                egs.append(eg)
            pend[i] = egs

        def emit_tail(i):
            s, ck = jobs[i]
            egs = pend.pop(i)
            vT = vT_l[s]
            cs = slice(ck * CHUNK, (ck + 1) * CHUNK)

            den = pWsm.tile([128, 4], f32, tag="sm")
            for j in range(4):
                for g in range(NPAIR):
                    nc.tensor.matmul(
                        den[:, j : j + 1],
                        egs[g][:, :, j * 128 : (j + 1) * 128],
                        on8[:],
                        start=(g == 0), stop=(g == NPAIR - 1),
                        perf_mode=DR,
                    )
            r4 = rp.tile([128, 4], f32, tag="r4")
            nc.vector.reciprocal_approx_fast(r4[:], den[:])
            prT = pWsm.tile([4, 128], f32, tag="sm")
            nc.tensor.transpose(prT[:], r4[:], idf[:])
            rr4 = rp.tile([4, 128], f32, tag="rr4")
            nc.vector.tensor_copy(rr4[:], prT[:])
            rrow = rp.tile([1, CHUNK], f32, tag="rrow")
            nc.gpsimd.dma_start(
                rrow[0:1, :].rearrange("o (j m) -> o j m", j=4), rr4[:]
            )
            rb = rp.tile([128, CHUNK], f32, tag="rb")
            nc.gpsimd.partition_broadcast(rb[:], rrow[0:1, :])

            pu = pWu.tile([128, CHUNK], f32, tag="u")
            for g in range(NPAIR):
                nc.tensor.matmul(
                    pu[:], vT[:, g, :, :], egs[g][:],
                    start=(g == 0), stop=(g == NPAIR - 1),
                    perf_mode=DR,
                )
            un = up.tile([128, CHUNK], e4, tag="un")
            nc.vector.tensor_mul(un[:], pu[:], rb[:])

            yt = yp.tile([128, 2, CHUNK], f32, tag="y")
            for mt in range(2):
                po = pWo.tile([128, CHUNK], f32, tag="o")
                nc.tensor.matmul(
                    po[:], wa[:, mt, :], un[:],
                    start=True, stop=True,
                )
                nc.vector.tensor_add(
                    yt[:, mt, :], po[:], xrs[s][:, mt, cs].bitcast(f32)
                )
            nc.sync.dma_start(
                y_d[s, :, :, cs].rearrange("t p m -> p t m"), yt[:]
            )

        for i in range(len(jobs) + LAG):
            if i < len(jobs):
                emit_head(i)
            if i >= LAG:
                emit_tail(i - LAG)

    nc.compile()
    return nc


def _get_program():
    if "nc" not in _built:
        _built["nc"] = _build_program()
    return _built["nc"]


def _make_in_maps(x, Wq, Wk, Wv, Wa, gamma):
    import ml_dtypes

    x = np.ascontiguousarray(
        np.asarray(x, dtype=np.float32).reshape(B, 2, 128, HWF)
    )
    wqkT = np.concatenate([np.asarray(Wq), np.asarray(Wk)], axis=0).T
    wqkT = np.ascontiguousarray(
        wqkT.reshape(2, 128, 64).transpose(1, 0, 2).astype(np.float32)
    )
    wvT = np.ascontiguousarray(
        np.asarray(Wv).T.reshape(2, 128, 128).transpose(1, 0, 2).astype(np.float32)
    )
    g = float(np.asarray(gamma).reshape(-1)[0])
    waT = np.ascontiguousarray(
        (g * np.asarray(Wa)).T.reshape(128, 2, 128).astype(ml_dtypes.bfloat16)
    )
    identB = np.eye(128, dtype=np.float32).astype(ml_dtypes.bfloat16)
    identF = np.eye(128, dtype=np.float32)
    ones8 = np.ones((128, 2, 1), dtype=ml_dtypes.float8_e5m2)
    kone = np.ones((1, KT, 128), dtype=ml_dtypes.bfloat16)
    return [
        {
            "x": np.ascontiguousarray(x[c * SPC : (c + 1) * SPC]),
            "wqkT": wqkT,
            "wvT": wvT,
            "waT": waT,
            "identB": identB,
            "identF": identF,
            "ones8": ones8,
            "kone": kone,
        }
        for c in range(NCORES)
    ]


def kernel(x, Wq, Wk, Wv, Wa, gamma):
    from concourse import bass_utils

    nc = _get_program()
    in_maps = _make_in_maps(x, Wq, Wk, Wv, Wa, gamma)
    res = bass_utils.run_bass_kernel_spmd(
        nc, in_maps, core_ids=list(range(NCORES))
    )
    out = np.concatenate(
        [res.results[c]["y"].reshape(1, SPC, C, HWF) for c in range(NCORES)],
        axis=0,
    ).reshape(B, C, H, W)
    return out


# revision 17
# speedup vs baseline: 1.3855x; 1.0218x over previous
"""Trainium2 Bass kernel for nn_Attention_5093831213465.

Reference computation (per sample, x_b: [256, 4096]):
  q = Wq @ x_b                       [32, 4096]
  k = maxpool2(Wk @ x_b)             [32, 1024]
  v = maxpool2(Wv @ x_b)             [128, 1024]
  attn = softmax_over_k(k^T @ q)     [1024, 4096]
  out  = Wa @ (v @ attn)             [256, 4096]
  y    = gamma * out + x_b
Sharding: data-parallel over batch, 2 samples per core on 8 cores.

Design (driven by the TimelineSim cost model):
- Matmul cost = out-free-size x cycles/row; fp8 DoubleRow = 0.5/row.
  The value matmul (v @ E) runs entirely in fp8 DoubleRow: E in e5m2
  written by the Act engine's exp, v^T in e4m3.
- Softmax denominators via "stationary-E": matmuls with E as the
  stationary operand and a ones column moving -> out free size 1, so
  the whole reduction costs ~nothing on the PE (vs. streaming E again).
- exp overflows e5m2 unless logits are shifted per column.  The shift
  rides the attention matmul as an extra contraction row: k row 32 is
  constant 1, q row 32 is -(submax[qq]+1), where submax is a 128-key
  subsampled column max computed by a small transposed attention
  (q-tile stationary) + a DVE free-dim max.  Measured gap between true
  colmax and 128-submax on this data is <= 8.81, safely under the
  ~12 overflow budget.
- Normalization happens pre-Wa on the DVE (un = psU * rb, e4m3 out);
  rb is built per chunk: denom -> reciprocal (bf16) -> PE transpose ->
  SBUF->SBUF partition-gather DMA -> gpsimd partition_broadcast.
  gpsimd cannot touch PSUM, so it only gets SBUF-only jobs.
- Residual adds on DVE from psO + x, one [128,2,512] instr per chunk.
- All large DMAs ride the SP queue; x loads are split so chunk 0's
  columns land first and the PE starts early.
"""

import sys

import numpy as np

if "/opt/trn_rl_repo" not in sys.path:
    sys.path.insert(0, "/opt/trn_rl_repo")

B, C, H, W = 16, 256, 64, 64
CA = C // 8          # 32  attn channels
CS = C // 2          # 128 value channels
HWF = H * W          # 4096 spatial positions
HWP = HWF // 4       # 1024 pooled positions
SPC = 2              # samples per core
NCORES = 8
CHUNK = 512
NCHUNK = HWF // CHUNK       # 8
KT = HWP // 128             # 8 kk tiles of 128
NPAIR = KT // 2             # 4 exp/U pairs per chunk
SHIFT_DELTA = 2.0           # c = submax + delta

_built = {}


def _build_program():
    from contextlib import ExitStack

    import concourse.bass as bass
    import concourse.tile as tile
    from concourse import bacc, mybir

    f32 = mybir.dt.float32
    f32r = mybir.dt.float32r
    bf16 = mybir.dt.bfloat16
    e4 = mybir.dt.float8e4
    e5 = mybir.dt.float8e5
    i16 = mybir.dt.int16
    DR = mybir.MatmulPerfMode.DoubleRow
    Exp = mybir.ActivationFunctionType.Exp
    Mult = mybir.AluOpType.mult
    Add = mybir.AluOpType.add
    Max = mybir.AluOpType.max

    nc = bacc.Bacc(
        "TRN2", target_bir_lowering=False, debug=False, enable_asserts=False
    )

    x_d = nc.dram_tensor("x", [SPC, 2, 128, HWF], f32r, kind="ExternalInput").ap()
    wqk_d = nc.dram_tensor("wqkT", [128, 2, 64], f32r, kind="ExternalInput").ap()
    wv_d = nc.dram_tensor("wvT", [128, 2, 128], f32r, kind="ExternalInput").ap()
    wa_d = nc.dram_tensor("waT", [128, 2, 128], bf16, kind="ExternalInput").ap()
    idb_d = nc.dram_tensor("identB", [128, 128], bf16, kind="ExternalInput").ap()
    idf_d = nc.dram_tensor("identF", [128, 128], f32, kind="ExternalInput").ap()
    on8_d = nc.dram_tensor("ones8", [128, 2, 1], e5, kind="ExternalInput").ap()
    kone_d = nc.dram_tensor("kone", [1, KT, 128], bf16, kind="ExternalInput").ap()
    onb_d = nc.dram_tensor("onesb", [128, 1], bf16, kind="ExternalInput").ap()
    y_d = nc.dram_tensor("y", [SPC, 2, 128, HWF], f32, kind="ExternalOutput").ap()

    with tile.TileContext(nc) as tc, ExitStack() as ctx:
        consts = ctx.enter_context(tc.tile_pool(name="consts", bufs=1))
        xp = ctx.enter_context(tc.tile_pool(name="xp", bufs=2))
        qsp = ctx.enter_context(tc.tile_pool(name="qsp", bufs=2))
        kvp = ctx.enter_context(tc.tile_pool(name="kvp", bufs=2))
        cm = ctx.enter_context(tc.tile_pool(name="cm", bufs=2))
        ep = ctx.enter_context(tc.tile_pool(name="ep", bufs=12))
        rp = ctx.enter_context(tc.tile_pool(name="rp", bufs=3))
        up = ctx.enter_context(tc.tile_pool(name="up", bufs=3))
        yp = ctx.enter_context(tc.tile_pool(name="yp", bufs=3))
        # PSUM budget (16KB/partition): pBig 2x[128,2,512]f32 (8KB) shared by
        # conv tiles and attn pairs (disjoint in time), pW 3x[128,512]f32 (6KB)
        # for U/rb-chain/Wa outputs and small transposes.
        pBig = ctx.enter_context(tc.tile_pool(name="pBig", bufs=2, space="PSUM"))
        pWu = ctx.enter_context(tc.tile_pool(name="pWu", bufs=2, space="PSUM"))
        pWo = ctx.enter_context(tc.tile_pool(name="pWo", bufs=1, space="PSUM"))
        pWsm = ctx.enter_context(tc.tile_pool(name="pWsm", bufs=1, space="PSUM"))

        wqk = consts.tile([128, 2, 64], f32r)
        nc.sync.dma_start(wqk[:], wqk_d)
        wv = consts.tile([128, 2, 128], f32r)
        nc.sync.dma_start(wv[:], wv_d)
        wa = consts.tile([128, 2, 128], bf16)
        nc.sync.dma_start(wa[:], wa_d)
        idb = consts.tile([128, 128], bf16)
        nc.sync.dma_start(idb[:], idb_d)
        idf = consts.tile([128, 128], f32)
        nc.sync.dma_start(idf[:], idf_d)
        on8 = consts.tile([128, 2, 1], e5)
        nc.sync.dma_start(on8[:], on8_d)
        onb = consts.tile([128, 1], bf16)
        nc.sync.dma_start(onb[:], onb_d)

        # x loads: front chunk first so conv starts early
        xrs = []
        for s in range(SPC):
            xr = xp.tile([128, 2, HWF], f32r, tag="xr")
            xrs.append(xr)
            for lo, hi in ((0, 512), (512, 2048), (2048, 4096)):
                for t in range(2):
                    nc.sync.dma_start(
                        xr[:, t, lo:hi], x_d[s, t, :, lo:hi]
                    )

        qs_l, kph_l, vT_l, vTb_l, cneg_l = [], [], [], [], []

        # ---- conv + pool + submax phases (both samples before attn) ----
        for s in range(SPC):
            qs = qsp.tile([33, KT, CHUNK], bf16, tag="qs")
            kph = kvp.tile([33, KT, 128], bf16, tag="kph")
            vph = kvp.tile([128, KT, 128], bf16, tag="vph")
            vT = kvp.tile([128, NPAIR, 2, 128], e4, tag="vT")
            vTb = kvp.tile([128, 2, 128], bf16, tag="vTb")
            qs_l.append(qs)
            kph_l.append(kph)
            vT_l.append(vT)
            vTb_l.append(vTb)

            # k-side ones row for the shift
            nc.sync.dma_start(kph[32:33, :, :], kone_d)

            for ck in range(NCHUNK):
                cs = slice(ck * CHUNK, (ck + 1) * CHUNK)
                pcv = pBig.tile([128, 2, CHUNK], f32, tag="big")
                for t in range(2):
                    nc.tensor.matmul(
                        pcv[0:64, 0, :], wqk[:, t, :], xrs[s][:, t, cs],
                        start=(t == 0), stop=(t == 1),
                    )
                nc.scalar.copy(qs[0:32, ck, :], pcv[0:32, 0, :])
                nc.vector.tensor_reduce(
                    kph[0:32, ck, :].rearrange("p (h2 w2) -> p h2 w2", h2=4),
                    pcv[32:64, 0, :].rearrange(
                        "p (h2 dh w2 dw) -> p h2 w2 dh dw", h2=4, dh=2, w2=32, dw=2
                    ),
                    axis=mybir.AxisListType.XY, op=Max,
                )
                for t in range(2):
                    nc.tensor.matmul(
                        pcv[:, 1, :], wv[:, t, :], xrs[s][:, t, cs],
                        start=(t == 0), stop=(t == 1),
                    )
                nc.vector.tensor_reduce(
                    vph[:, ck, :].rearrange("p (h2 w2) -> p h2 w2", h2=4),
                    pcv[:, 1, :].rearrange(
                        "p (h2 dh w2 dw) -> p h2 w2 dh dw", h2=4, dh=2, w2=32, dw=2
                    ),
                    axis=mybir.AxisListType.XY, op=Max,
                )
                ptr = pWsm.tile([128, 128], bf16, tag="sm")
                nc.tensor.transpose(ptr[:], vph[:, ck, :], idb[:])
                if ck < KT - 2:
                    nc.scalar.copy(vT[:, ck // 2, ck % 2, :], ptr[:])
                else:
                    nc.scalar.copy(vTb[:, ck % 2, :], ptr[:])

            # submax: transposed 128-key subsampled attention + free max.
            # bf16 moving operand: f32r would pay the 4x short-row penalty
            # on the [*, 128] outputs.
            ksub = kph[0:32, :, :].rearrange(
                "p kt (j v) -> p kt j v", v=16
            )[:, :, :, 0]
            cmax = cm.tile([128, 32], f32r, tag="cmax")
            for ck in range(NCHUNK):
                psm = pBig.tile([128, 4, 64], f32, tag="big")
                for j in range(4):
                    nc.tensor.matmul(
                        psm[:, j, :],
                        qs[0:32, ck, j * 128 : (j + 1) * 128],
                        ksub,
                        start=True, stop=True,
                    )
                nc.vector.tensor_reduce(
                    cmax[:, ck * 4 : ck * 4 + 4],
                    psm[:],
                    axis=mybir.AxisListType.X, op=Max,
                )
            cneg = cm.tile([128, 32], bf16, tag="cneg")
            nc.vector.tensor_scalar(
                cneg[:], cmax[:], -1.0, -SHIFT_DELTA, Mult, Add
            )
            pcn = pWsm.tile([32, 128], bf16, tag="sm")
            nc.tensor.transpose(pcn[:], cneg[:], idb[:])
            cnT = cm.tile([32, 128], bf16, tag="cnT")
            nc.vector.tensor_copy(cnT[:], pcn[:])
            cneg_l.append(cnT)
            # scatter the q shift row: [32,128] partitions -> [1, 8, 512]
            nc.gpsimd.dma_start(
                qs[32:33, :, :].rearrange("o kt (j m) -> o (kt j) m", j=4),
                cnT[:],
            )

        # ---- attention phases: 2-chunk software pipeline ----
        # PE executes in order, so chunk tails (denominator chain, U, Wa)
        # that wait on the Act exp stream are emitted two chunks behind the
        # attention pair matmuls; the PE never blocks on exp.
        jobs = [(s, ck) for s in range(SPC) for ck in range(NCHUNK)]
        LAG = 2
        pend = {}

        def emit_head(i):
            s, ck = jobs[i]
            qs, kph = qs_l[s], kph_l[s]
            egs = []
            for g in range(NPAIR):
                pa = pBig.tile([128, 2, CHUNK], f32, tag="big")
                for t in range(2):
                    nc.tensor.matmul(
                        pa[:, t, :],
                        kph[:, 2 * g + t, :],
                        qs[:, ck, :],
                        start=True, stop=True,
                    )
                if g < NPAIR - 1:
                    eg = ep.tile([128, 2, CHUNK], e5, tag="E")
                    nc.scalar.activation(eg[:], pa[:], Exp)
                else:
                    # Schraudolph exp in bf16 bits on the DVE:
                    # bits16 = rne(z*184.665 + 16250.5); bitcast -> bf16
                    eg = ep.tile([128, 2, CHUNK], i16, tag="E")
                    nc.vector.tensor_scalar(
                        eg[:], pa[:], 184.6650, 16250.5, Mult, Add
                    )
                    eg = eg.bitcast(bf16)
                egs.append(eg)
            pend[i] = egs

        def emit_tail(i):
            s, ck = jobs[i]
            egs = pend.pop(i)
            vT = vT_l[s]
            cs = slice(ck * CHUNK, (ck + 1) * CHUNK)

            den = pWsm.tile([128, 4], f32, tag="sm")
            for j in range(4):
                for g in range(NPAIR - 1):
                    nc.tensor.matmul(
                        den[:, j : j + 1],
                        egs[g][:, :, j * 128 : (j + 1) * 128],
                        on8[:],
                        start=(g == 0), stop=False,
                        perf_mode=DR,
                    )
                for t in range(2):
                    nc.tensor.matmul(
                        den[:, j : j + 1],
                        egs[NPAIR - 1][:, t, j * 128 : (j + 1) * 128],
                        onb[:],
                        start=False, stop=(t == 1),
                    )
            r4 = rp.tile([128, 4], f32, tag="r4")
            nc.vector.reciprocal_approx_fast(r4[:], den[:])
            prT = pWsm.tile([4, 128], f32, tag="sm")
            nc.tensor.transpose(prT[:], r4[:], idf[:])
            rr4 = rp.tile([4, 128], f32, tag="rr4")
            nc.vector.tensor_copy(rr4[:], prT[:])
            rrow = rp.tile([1, CHUNK], f32, tag="rrow")
            nc.gpsimd.dma_start(
                rrow[0:1, :].rearrange("o (j m) -> o j m", j=4), rr4[:]
            )
            rb = rp.tile([128, CHUNK], f32, tag="rb")
            nc.gpsimd.partition_broadcast(rb[:], rrow[0:1, :])

            pu = pWu.tile([128, CHUNK], f32, tag="u")
            for g in range(NPAIR - 1):
                nc.tensor.matmul(
                    pu[:], vT[:, g, :, :], egs[g][:],
                    start=(g == 0), stop=False,
                    perf_mode=DR,
                )
            vTb = vTb_l[s]
            for t in range(2):
                nc.tensor.matmul(
                    pu[:], vTb[:, t, :], egs[NPAIR - 1][:, t, :],
                    start=False, stop=(t == 1),
                )
            un = up.tile([128, CHUNK], e4, tag="un")
            nc.vector.tensor_mul(un[:], pu[:], rb[:])

            yt = yp.tile([128, 2, CHUNK], f32, tag="y")
            for mt in range(2):
                po = pWo.tile([128, CHUNK], f32, tag="o")
                nc.tensor.matmul(
                    po[:], wa[:, mt, :], un[:],
                    start=True, stop=True,
                )
                nc.vector.tensor_add(
                    yt[:, mt, :], po[:], xrs[s][:, mt, cs].bitcast(f32)
                )
            nc.sync.dma_start(
                y_d[s, :, :, cs].rearrange("t p m -> p t m"), yt[:]
            )

        for i in range(len(jobs) + LAG):
            if i < len(jobs):
                emit_head(i)
            if i >= LAG:
                emit_tail(i - LAG)

    nc.compile()
    return nc


def _get_program():
    if "nc" not in _built:
        _built["nc"] = _build_program()
    return _built["nc"]


def _make_in_maps(x, Wq, Wk, Wv, Wa, gamma):
    import ml_dtypes

    x = np.ascontiguousarray(
        np.asarray(x, dtype=np.float32).reshape(B, 2, 128, HWF)
    )
    wqkT = np.concatenate([np.asarray(Wq), np.asarray(Wk)], axis=0).T
    wqkT = np.ascontiguousarray(
        wqkT.reshape(2, 128, 64).transpose(1, 0, 2).astype(np.float32)
    )
    wvT = np.ascontiguousarray(
        np.asarray(Wv).T.reshape(2, 128, 128).transpose(1, 0, 2).astype(np.float32)
    )
    g = float(np.asarray(gamma).reshape(-1)[0])
    waT = np.ascontiguousarray(
        (g * np.asarray(Wa)).T.reshape(128, 2, 128).astype(ml_dtypes.bfloat16)
    )
    identB = np.eye(128, dtype=np.float32).astype(ml_dtypes.bfloat16)
    identF = np.eye(128, dtype=np.float32)
    ones8 = np.ones((128, 2, 1), dtype=ml_dtypes.float8_e5m2)
    kone = np.ones((1, KT, 128), dtype=ml_dtypes.bfloat16)
    return [
        {
            "x": np.ascontiguousarray(x[c * SPC : (c + 1) * SPC]),
            "wqkT": wqkT,
            "wvT": wvT,
            "waT": waT,
            "identB": identB,
            "identF": identF,
            "ones8": ones8,
            "onesb": np.ones((128, 1), dtype=ml_dtypes.bfloat16),
            "kone": kone,
        }
        for c in range(NCORES)
    ]


def kernel(x, Wq, Wk, Wv, Wa, gamma):
    from concourse import bass_utils

    nc = _get_program()
    in_maps = _make_in_maps(x, Wq, Wk, Wv, Wa, gamma)
    res = bass_utils.run_bass_kernel_spmd(
        nc, in_maps, core_ids=list(range(NCORES))
    )
    out = np.concatenate(
        [res.results[c]["y"].reshape(1, SPC, C, HWF) for c in range(NCORES)],
        axis=0,
    ).reshape(B, C, H, W)
    return out


# revision 23
# speedup vs baseline: 1.3962x; 1.0077x over previous
"""Trainium2 Bass kernel for nn_Attention_5093831213465.

Reference computation (per sample, x_b: [256, 4096]):
  q = Wq @ x_b                       [32, 4096]
  k = maxpool2(Wk @ x_b)             [32, 1024]
  v = maxpool2(Wv @ x_b)             [128, 1024]
  attn = softmax_over_k(k^T @ q)     [1024, 4096]
  out  = Wa @ (v @ attn)             [256, 4096]
  y    = gamma * out + x_b
Sharding: data-parallel over batch, 2 samples per core on 8 cores.

Design (driven by the TimelineSim cost model):
- Matmul cost = out-free-size x cycles/row; fp8 DoubleRow = 0.5/row.
  The value matmul (v @ E) runs entirely in fp8 DoubleRow: E in e5m2
  written by the Act engine's exp, v^T in e4m3.
- Softmax denominators via "stationary-E": matmuls with E as the
  stationary operand and a ones column moving -> out free size 1, so
  the whole reduction costs ~nothing on the PE (vs. streaming E again).
- exp overflows e5m2 unless logits are shifted per column.  The shift
  rides the attention matmul as an extra contraction row: k row 32 is
  constant 1, q row 32 is -(submax[qq]+1), where submax is a 128-key
  subsampled column max computed by a small transposed attention
  (q-tile stationary) + a DVE free-dim max.  Measured gap between true
  colmax and 128-submax on this data is <= 8.81, safely under the
  ~12 overflow budget.
- Normalization happens pre-Wa on the DVE (un = psU * rb, e4m3 out);
  rb is built per chunk: denom -> reciprocal (bf16) -> PE transpose ->
  SBUF->SBUF partition-gather DMA -> gpsimd partition_broadcast.
  gpsimd cannot touch PSUM, so it only gets SBUF-only jobs.
- Residual adds on DVE from psO + x, one [128,2,512] instr per chunk.
- All large DMAs ride the SP queue; x loads are split so chunk 0's
  columns land first and the PE starts early.
"""

import sys

import numpy as np

if "/opt/trn_rl_repo" not in sys.path:
    sys.path.insert(0, "/opt/trn_rl_repo")

B, C, H, W = 16, 256, 64, 64
CA = C // 8          # 32  attn channels
CS = C // 2          # 128 value channels
HWF = H * W          # 4096 spatial positions
HWP = HWF // 4       # 1024 pooled positions
SPC = 2              # samples per core
NCORES = 8
CHUNK = 512
NCHUNK = HWF // CHUNK       # 8
KT = HWP // 128             # 8 kk tiles of 128
NPAIR = KT // 2             # 4 exp/U pairs per chunk
SHIFT_DELTA = 2.0           # c = submax + delta

_built = {}


def _build_program():
    from contextlib import ExitStack

    import concourse.bass as bass
    import concourse.tile as tile
    from concourse import bacc, mybir

    f32 = mybir.dt.float32
    f32r = mybir.dt.float32r
    bf16 = mybir.dt.bfloat16
    e4 = mybir.dt.float8e4
    e5 = mybir.dt.float8e5
    i16 = mybir.dt.int16
    DR = mybir.MatmulPerfMode.DoubleRow
    Exp = mybir.ActivationFunctionType.Exp
    Mult = mybir.AluOpType.mult
    Add = mybir.AluOpType.add
    Max = mybir.AluOpType.max

    nc = bacc.Bacc(
        "TRN2", target_bir_lowering=False, debug=False, enable_asserts=False
    )

    x_d = nc.dram_tensor("x", [SPC, 2, 128, HWF], f32r, kind="ExternalInput").ap()
    wqk_d = nc.dram_tensor("wqkT", [128, 2, 64], f32r, kind="ExternalInput").ap()
    wv_d = nc.dram_tensor("wvT", [128, 2, 128], f32r, kind="ExternalInput").ap()
    wa_d = nc.dram_tensor("waT", [128, 2, 128], bf16, kind="ExternalInput").ap()
    idb_d = nc.dram_tensor("identB", [128, 128], bf16, kind="ExternalInput").ap()
    idf_d = nc.dram_tensor("identF", [128, 128], f32, kind="ExternalInput").ap()
    on8_d = nc.dram_tensor("ones8", [128, 2, 1], e5, kind="ExternalInput").ap()
    kone_d = nc.dram_tensor("kone", [1, KT, 128], bf16, kind="ExternalInput").ap()
    onb_d = nc.dram_tensor("onesb", [128, 1], bf16, kind="ExternalInput").ap()
    y_d = nc.dram_tensor("y", [SPC, 2, 128, HWF], f32, kind="ExternalOutput").ap()

    with tile.TileContext(nc) as tc, ExitStack() as ctx:
        consts = ctx.enter_context(tc.tile_pool(name="consts", bufs=1))
        xp = ctx.enter_context(tc.tile_pool(name="xp", bufs=2))
        qsp = ctx.enter_context(tc.tile_pool(name="qsp", bufs=2))
        kvp = ctx.enter_context(tc.tile_pool(name="kvp", bufs=2))
        cm = ctx.enter_context(tc.tile_pool(name="cm", bufs=2))
        plp = ctx.enter_context(tc.tile_pool(name="plp", bufs=3))
        ep = ctx.enter_context(tc.tile_pool(name="ep", bufs=12))
        rp = ctx.enter_context(tc.tile_pool(name="rp", bufs=3))
        up = ctx.enter_context(tc.tile_pool(name="up", bufs=3))
        yp = ctx.enter_context(tc.tile_pool(name="yp", bufs=3))
        # PSUM budget (16KB/partition): pBig 2x[128,2,512]f32 (8KB) shared by
        # conv tiles and attn pairs (disjoint in time), pW 3x[128,512]f32 (6KB)
        # for U/rb-chain/Wa outputs and small transposes.
        pBig = ctx.enter_context(tc.tile_pool(name="pBig", bufs=2, space="PSUM"))
        pWu = ctx.enter_context(tc.tile_pool(name="pWu", bufs=2, space="PSUM"))
        pWo = ctx.enter_context(tc.tile_pool(name="pWo", bufs=1, space="PSUM"))
        pWsm = ctx.enter_context(tc.tile_pool(name="pWsm", bufs=1, space="PSUM"))

        wqk = consts.tile([128, 2, 64], f32r)
        nc.sync.dma_start(wqk[:], wqk_d)
        wv = consts.tile([128, 2, 128], f32r)
        nc.sync.dma_start(wv[:], wv_d)
        wa = consts.tile([128, 2, 128], bf16)
        nc.sync.dma_start(wa[:], wa_d)
        idb = consts.tile([128, 128], bf16)
        nc.sync.dma_start(idb[:], idb_d)
        idf = consts.tile([128, 128], f32)
        nc.sync.dma_start(idf[:], idf_d)
        on8 = consts.tile([128, 2, 1], e5)
        nc.sync.dma_start(on8[:], on8_d)
        onb = consts.tile([128, 1], bf16)
        nc.sync.dma_start(onb[:], onb_d)

        # x loads: front chunk first so conv starts early
        xrs = []
        for s in range(SPC):
            xr = xp.tile([128, 2, HWF], f32r, tag="xr")
            xrs.append(xr)
            for lo, hi in ((0, 512), (512, 2048), (2048, 4096)):
                for t in range(2):
                    nc.sync.dma_start(
                        xr[:, t, lo:hi], x_d[s, t, :, lo:hi]
                    )

        qs_l, kph_l, vT_l, vTb_l, cneg_l = [], [], [], [], []

        # ---- conv + pool + submax phases (both samples before attn) ----
        for s in range(SPC):
            qs = qsp.tile([33, KT, CHUNK], bf16, tag="qs")
            kph = kvp.tile([33, KT, 128], bf16, tag="kph")
            vph = kvp.tile([128, KT, 128], bf16, tag="vph")
            vT = kvp.tile([128, NPAIR, 2, 128], e4, tag="vT")
            vTb = kvp.tile([128, 2, 128], bf16, tag="vTb")
            qs_l.append(qs)
            kph_l.append(kph)
            vT_l.append(vT)
            vTb_l.append(vTb)

            # k-side ones row for the shift
            nc.sync.dma_start(kph[32:33, :, :], kone_d)

            for ck in range(NCHUNK):
                cs = slice(ck * CHUNK, (ck + 1) * CHUNK)
                pcv = pBig.tile([128, 2, CHUNK], f32, tag="big")
                for t in range(2):
                    nc.tensor.matmul(
                        pcv[0:64, 0, :], wqk[:, t, :], xrs[s][:, t, cs],
                        start=(t == 0), stop=(t == 1),
                    )
                nc.scalar.copy(qs[0:32, ck, :], pcv[0:32, 0, :])
                nc.vector.tensor_reduce(
                    kph[0:32, ck, :].rearrange("p (h2 w2) -> p h2 w2", h2=4),
                    pcv[32:64, 0, :].rearrange(
                        "p (h2 dh w2 dw) -> p h2 w2 dh dw", h2=4, dh=2, w2=32, dw=2
                    ),
                    axis=mybir.AxisListType.XY, op=Max,
                )
                for t in range(2):
                    nc.tensor.matmul(
                        pcv[:, 1, :], wv[:, t, :], xrs[s][:, t, cs],
                        start=(t == 0), stop=(t == 1),
                    )
                nc.vector.tensor_reduce(
                    vph[:, ck, :].rearrange("p (h2 w2) -> p h2 w2", h2=4),
                    pcv[:, 1, :].rearrange(
                        "p (h2 dh w2 dw) -> p h2 w2 dh dw", h2=4, dh=2, w2=32, dw=2
                    ),
                    axis=mybir.AxisListType.XY, op=Max,
                )
                ptr = pWsm.tile([128, 128], bf16, tag="sm")
                nc.tensor.transpose(ptr[:], vph[:, ck, :], idb[:])
                nc.scalar.copy(vT[:, ck // 2, ck % 2, :], ptr[:])
                if ck >= KT - 2:
                    nc.scalar.copy(vTb[:, ck % 2, :], ptr[:])

            # submax: transposed 128-key subsampled attention + free max.
            # bf16 moving operand: f32r would pay the 4x short-row penalty
            # on the [*, 128] outputs.
            ksub = kph[0:32, :, :].rearrange(
                "p kt (j v) -> p kt j v", v=16
            )[:, :, :, 0]
            cmax = cm.tile([128, 32], f32r, tag="cmax")
            for ck in range(NCHUNK):
                psm = pBig.tile([128, 4, 64], f32, tag="big")
                for j in range(4):
                    nc.tensor.matmul(
                        psm[:, j, :],
                        qs[0:32, ck, j * 128 : (j + 1) * 128],
                        ksub,
                        start=True, stop=True,
                    )
                nc.vector.tensor_reduce(
                    cmax[:, ck * 4 : ck * 4 + 4],
                    psm[:],
                    axis=mybir.AxisListType.X, op=Max,
                )
            cneg = cm.tile([128, 32], bf16, tag="cneg")
            nc.vector.tensor_scalar(
                cneg[:], cmax[:], -1.0, -SHIFT_DELTA, Mult, Add
            )
            pcn = pWsm.tile([32, 128], bf16, tag="sm")
            nc.tensor.transpose(pcn[:], cneg[:], idb[:])
            cnT = cm.tile([32, 128], bf16, tag="cnT")
            nc.vector.tensor_copy(cnT[:], pcn[:])
            cneg_l.append(cnT)
            # scatter the q shift row: [32,128] partitions -> [1, 8, 512]
            nc.gpsimd.dma_start(
                qs[32:33, :, :].rearrange("o kt (j m) -> o (kt j) m", j=4),
                cnT[:],
            )

        # ---- attention phases: 2-chunk software pipeline ----
        # PE executes in order, so chunk tails (denominator chain, U, Wa)
        # that wait on the Act exp stream are emitted two chunks behind the
        # attention pair matmuls; the PE never blocks on exp.
        jobs = [(s, ck) for s in range(SPC) for ck in range(NCHUNK)]
        LAG = 2
        pend = {}

        def emit_head(i):
            s, ck = jobs[i]
            qs, kph = qs_l[s], kph_l[s]
            egs = []
            for g in range(NPAIR):
                pa = pBig.tile([128, 2, CHUNK], f32, tag="big")
                for t in range(2):
                    nc.tensor.matmul(
                        pa[:, t, :],
                        kph[:, 2 * g + t, :],
                        qs[:, ck, :],
                        start=True, stop=True,
                    )
                if g < NPAIR - 1 or (i % 3 == 0):
                    eg = ep.tile([128, 2, CHUNK], e5, tag="E")
                    nc.scalar.activation(eg[:], pa[:], Exp)
                else:
                    # Schraudolph exp in bf16 bits on the DVE:
                    # bits16 = rne(z*184.665 + 16250.5); bitcast -> bf16
                    eg = ep.tile([128, 2, CHUNK], i16, tag="E")
                    nc.vector.tensor_scalar(
                        eg[:], pa[:], 184.6650, 16250.5, Mult, Add
                    )
                    eg = eg.bitcast(bf16)
                egs.append(eg)
            pend[i] = (egs, i % 3 != 0)

        def emit_tail(i):
            s, ck = jobs[i]
            egs, last_bf = pend.pop(i)
            vT = vT_l[s]
            cs = slice(ck * CHUNK, (ck + 1) * CHUNK)

            den = pWsm.tile([128, 4], f32, tag="sm")
            for j in range(4):
                for g in range(NPAIR - 1):
                    nc.tensor.matmul(
                        den[:, j : j + 1],
                        egs[g][:, :, j * 128 : (j + 1) * 128],
                        on8[:],
                        start=(g == 0), stop=False,
                        perf_mode=DR,
                    )
                if last_bf:
                    for t in range(2):
                        nc.tensor.matmul(
                            den[:, j : j + 1],
                            egs[NPAIR - 1][:, t, j * 128 : (j + 1) * 128],
                            onb[:],
                            start=False, stop=(t == 1),
                        )
                else:
                    nc.tensor.matmul(
                        den[:, j : j + 1],
                        egs[NPAIR - 1][:, :, j * 128 : (j + 1) * 128],
                        on8[:],
                        start=False, stop=True,
                        perf_mode=DR,
                    )
            r4 = rp.tile([128, 4], f32, tag="r4")
            nc.vector.reciprocal_approx_fast(r4[:], den[:])
            prT = pWsm.tile([4, 128], f32, tag="sm")
            nc.tensor.transpose(prT[:], r4[:], idf[:])
            rr4 = rp.tile([4, 128], f32, tag="rr4")
            nc.vector.tensor_copy(rr4[:], prT[:])
            rrow = rp.tile([1, CHUNK], f32, tag="rrow")
            nc.gpsimd.dma_start(
                rrow[0:1, :].rearrange("o (j m) -> o j m", j=4), rr4[:]
            )
            rb = rp.tile([128, CHUNK], f32, tag="rb")
            nc.gpsimd.partition_broadcast(rb[:], rrow[0:1, :])

            pu = pWu.tile([128, CHUNK], f32, tag="u")
            for g in range(NPAIR - 1):
                nc.tensor.matmul(
                    pu[:], vT[:, g, :, :], egs[g][:],
                    start=(g == 0), stop=False,
                    perf_mode=DR,
                )
            if last_bf:
                vTb = vTb_l[s]
                for t in range(2):
                    nc.tensor.matmul(
                        pu[:], vTb[:, t, :], egs[NPAIR - 1][:, t, :],
                        start=False, stop=(t == 1),
                    )
            else:
                nc.tensor.matmul(
                    pu[:], vT[:, NPAIR - 1, :, :], egs[NPAIR - 1][:],
                    start=False, stop=True,
                    perf_mode=DR,
                )
            un = up.tile([128, CHUNK], e4, tag="un")
            nc.vector.tensor_mul(un[:], pu[:], rb[:])

            yt = yp.tile([128, 2, CHUNK], f32, tag="y")
            for mt in range(2):
                po = pWo.tile([128, CHUNK], f32, tag="o")
                nc.tensor.matmul(
                    po[:], wa[:, mt, :], un[:],
                    start=True, stop=True,
                )
                nc.vector.tensor_add(
                    yt[:, mt, :], po[:], xrs[s][:, mt, cs].bitcast(f32)
                )
            nc.sync.dma_start(
                y_d[s, :, :, cs].rearrange("t p m -> p t m"), yt[:]
            )

        for i in range(len(jobs) + LAG):
            if i < len(jobs):
                emit_head(i)
            if i >= LAG:
                emit_tail(i - LAG)

    nc.compile()
    return nc


def _get_program():
    if "nc" not in _built:
        _built["nc"] = _build_program()
    return _built["nc"]


def _make_in_maps(x, Wq, Wk, Wv, Wa, gamma):
    import ml_dtypes

    x = np.ascontiguousarray(
        np.asarray(x, dtype=np.float32).reshape(B, 2, 128, HWF)
    )
    wqkT = np.concatenate([np.asarray(Wq), np.asarray(Wk)], axis=0).T
    wqkT = np.ascontiguousarray(
        wqkT.reshape(2, 128, 64).transpose(1, 0, 2).astype(np.float32)
    )
    wvT = np.ascontiguousarray(
        np.asarray(Wv).T.reshape(2, 128, 128).transpose(1, 0, 2).astype(np.float32)
    )
    g = float(np.asarray(gamma).reshape(-1)[0])
    waT = np.ascontiguousarray(
        (g * np.asarray(Wa)).T.reshape(128, 2, 128).astype(ml_dtypes.bfloat16)
    )
    identB = np.eye(128, dtype=np.float32).astype(ml_dtypes.bfloat16)
    identF = np.eye(128, dtype=np.float32)
    ones8 = np.ones((128, 2, 1), dtype=ml_dtypes.float8_e5m2)
    kone = np.ones((1, KT, 128), dtype=ml_dtypes.bfloat16)
    return [
        {
            "x": np.ascontiguousarray(x[c * SPC : (c + 1) * SPC]),
            "wqkT": wqkT,
            "wvT": wvT,
            "waT": waT,
            "identB": identB,
            "identF": identF,
            "ones8": ones8,
            "onesb": np.ones((128, 1), dtype=ml_dtypes.bfloat16),
            "kone": kone,
        }
        for c in range(NCORES)
    ]


def kernel(x, Wq, Wk, Wv, Wa, gamma):
    from concourse import bass_utils

    nc = _get_program()
    in_maps = _make_in_maps(x, Wq, Wk, Wv, Wa, gamma)
    res = bass_utils.run_bass_kernel_spmd(
        nc, in_maps, core_ids=list(range(NCORES))
    )
    out = np.concatenate(
        [res.results[c]["y"].reshape(1, SPC, C, HWF) for c in range(NCORES)],
        axis=0,
    ).reshape(B, C, H, W)
    return out
